# revision 3
# baseline (speedup 1.0000x reference)
"""Fused 7-gate continuous-time LSTM cell on 8 Trainium2 NeuronCores.

Data-parallel over the batch dim: each core gets B/8 = 1024 rows, the fused
gate weight W [2048, 7*2048] is replicated (bf16). Per core:
  g = hx @ W + b  with hx/W in bf16 (PE streams 1 col/cycle, FWL hides
  LDWEIGHTS), accumulated fp32 in PSUM.

Layout: W columns are host-permuted to [hb, gate, 256] so each (m-chunk,
hb-block) accumulates all 7 gates' 256 h-columns contiguously (1792 cols =
2 PSUM tiles), letting the epilogue run right after each block.

Epilogue uses only Exp/Ln (single ACT table set, no ~2.7us table switches):
  sigmoid(x) = exp(-softplus(-x)) = Exp(-Ln(1+Exp(-x)))
  tanh(x)    = 2*sigmoid(2x) - 1
  E          = exp(-u*softplus(d)) via Exp(scale=-u[partition]) on Ln(1+Exp(d))
The 5 sigmoid gates are batched as one 1280-wide ACT chain.
"""

import sys

sys.path.insert(0, "/opt/trn_rl_repo")

import numpy as np
import ml_dtypes

import concourse.bass as bass
import concourse.mybir as mybir
import concourse.tile as tile
from concourse import bacc, bass_utils

B, D, H, NG = 8192, 2048, 2048, 7
N_CORES = 8
BL = B // N_CORES  # 1024 rows per core
P = 128
HB = 256  # h-block per epilogue step
N_HB = H // HB  # 8
GW = NG * HB  # 1792 gate-block width per hb
KT = D // P  # 16 contraction subtiles
MT = BL // P  # 8 m-chunks per core
# matmul column slices inside a (m, hb) block: 2 PSUM tiles of [128, 1024]
MM_SLICES = [(0, 0, 512), (0, 512, 512), (1, 0, 512), (1, 512, 256)]

F32 = mybir.dt.float32
BF16 = mybir.dt.bfloat16
AF = mybir.ActivationFunctionType
BF16NP = ml_dtypes.bfloat16

_cached_nc = None


def _build():
    nc = bacc.Bacc("TRN2", target_bir_lowering=False, debug=False,
                   num_devices=N_CORES)
    hxT = nc.dram_tensor("hxT", [D, BL], BF16, kind="ExternalInput").ap()
    cx1 = nc.dram_tensor("cx1", [BL, H], F32, kind="ExternalInput").ap()
    cx2 = nc.dram_tensor("cx2", [BL, H], F32, kind="ExternalInput").ap()
    dt_in = nc.dram_tensor("dt", [BL, 1], F32, kind="ExternalInput").ap()
    Wp = nc.dram_tensor("Wp", [D, NG * H], BF16, kind="ExternalInput").ap()
    bp = nc.dram_tensor("bp", [N_HB, GW], BF16, kind="ExternalInput").ap()
    out = nc.dram_tensor("out", [3, BL, H], F32, kind="ExternalOutput").ap()

    from contextlib import ExitStack

    with tile.TileContext(nc) as tc, ExitStack() as ctx:
        const_pool = ctx.enter_context(tc.tile_pool(name="const", bufs=1))
        hx_pool = ctx.enter_context(tc.tile_pool(name="hx", bufs=1))
        small_pool = ctx.enter_context(tc.tile_pool(name="small", bufs=4))
        wpool = ctx.enter_context(tc.tile_pool(name="w", bufs=18))
        bias_pool = ctx.enter_context(tc.tile_pool(name="bias", bufs=2))
        psum_pool = ctx.enter_context(tc.tile_pool(name="ps", bufs=4, space="PSUM"))
        gsb_pool = ctx.enter_context(tc.tile_pool(name="gsb", bufs=2))
        sgt_pool = ctx.enter_context(tc.tile_pool(name="sgt", bufs=2))
        sig_pool = ctx.enter_context(tc.tile_pool(name="sig", bufs=2))
        epi_pool = ctx.enter_context(tc.tile_pool(name="epi", bufs=2))
        cx_pool = ctx.enter_context(tc.tile_pool(name="cx", bufs=4))
        out_pool = ctx.enter_context(tc.tile_pool(name="outp", bufs=2))

        # -u per batch row (u = dt), laid out [128, m-chunk]
        negu = const_pool.tile([P, MT], F32)
        for m in range(MT):
            dtt = small_pool.tile([P, 1], F32, tag="dt")
            nc.sync.dma_start(dtt, dt_in[m * P : (m + 1) * P, :])
            nc.vector.tensor_scalar_mul(negu[:, m : m + 1], dtt, -1.0)

        # resident hx^T in bf16: [d-partition, k-subtile, batch-col]
        hxT_sb = hx_pool.tile([P, KT, BL], BF16)
        for k in range(KT):
            nc.gpsimd.dma_start(hxT_sb[:, k, :], hxT[k * P : (k + 1) * P, :])

        for hb in range(N_HB):
            # bias block for this hb, broadcast to 128 partitions
            bsl = bp[hb, :]
            b_bcast = bass.AP(
                tensor=bsl.tensor, offset=bsl.offset, ap=[[0, P], *bsl.ap]
            )
            bt = bias_pool.tile([P, GW], BF16, tag="bt")
            nc.gpsimd.dma_start(bt, b_bcast)

            wts = []
            for k in range(KT):
                wt = wpool.tile([P, GW], BF16, tag="w", name=f"w_{hb}_{k}")
                nc.sync.dma_start(
                    wt, Wp[k * P : (k + 1) * P, hb * GW : (hb + 1) * GW]
                )
                wts.append(wt)

            for m in range(MT):
                ms = slice(m * P, (m + 1) * P)
                cs = slice(hb * HB, (hb + 1) * HB)
                cx1t = cx_pool.tile([P, HB], F32, tag="cx1")
                nc.gpsimd.dma_start(cx1t, cx1[ms, cs])
                cx2t = cx_pool.tile([P, HB], F32, tag="cx2")
                nc.gpsimd.dma_start(cx2t, cx2[ms, cs])

                ps0 = psum_pool.tile([P, 1024], F32, tag="ps", name=f"ps0_{hb}_{m}")
                ps1 = psum_pool.tile([P, 1024], F32, tag="ps", name=f"ps1_{hb}_{m}")
                pst = (ps0, ps1)
                for k in range(KT):
                    lhs = hxT_sb[:, k, ms]
                    for ti, c0, w in MM_SLICES:
                        wc0 = ti * 1024 + c0
                        nc.tensor.matmul(
                            pst[ti][:, c0 : c0 + w],
                            lhs,
                            wts[k][:, wc0 : wc0 + w],
                            start=(k == 0),
                            stop=(k == KT - 1),
                        )

                # ---- epilogue for this [128 rows, 256 h-cols] block ----
                gsb = gsb_pool.tile([P, GW], F32, tag="gsb")
                nc.vector.tensor_add(gsb[:, 0:1024], ps0[:], bt[:, 0:1024])
                nc.vector.tensor_add(gsb[:, 1024:GW], ps1[:, 0:768],
                                     bt[:, 1024:GW])

                # 5 sigmoid gates (i1,i2,f1,f2,o) batched 1280 wide:
                # sig = Exp(-Ln(1 + Exp(-x)))
                e1 = sgt_pool.tile([P, 5 * HB], F32, tag="sgt")
                nc.scalar.activation(e1, gsb[:, 0 : 5 * HB], AF.Exp, scale=-1.0)
                sp = sgt_pool.tile([P, 5 * HB], F32, tag="sgt")
                nc.scalar.activation(sp, e1, AF.Ln, bias=1.0)
                sig = sig_pool.tile([P, 5 * HB], F32, tag="sig")
                nc.scalar.activation(sig, sp, AF.Exp, scale=-1.0)
                i1 = sig[:, 0:HB]
                i2 = sig[:, HB : 2 * HB]
                f1 = sig[:, 2 * HB : 3 * HB]
                f2 = sig[:, 3 * HB : 4 * HB]
                o = sig[:, 4 * HB : 5 * HB]

                # z = tanh(x) = 2*sigmoid(2x) - 1
                ez = epi_pool.tile([P, HB], F32, tag="ez")
                nc.scalar.activation(ez, gsb[:, 5 * HB : 6 * HB], AF.Exp,
                                     scale=-2.0)
                spz = epi_pool.tile([P, HB], F32, tag="spz")
                nc.scalar.activation(spz, ez, AF.Ln, bias=1.0)
                s2z = epi_pool.tile([P, HB], F32, tag="s2z")
                nc.scalar.activation(s2z, spz, AF.Exp, scale=-1.0)
                z = epi_pool.tile([P, HB], F32, tag="z")
                nc.vector.tensor_scalar(
                    out=z, in0=s2z, scalar1=2.0, scalar2=-1.0,
                    op0=mybir.AluOpType.mult, op1=mybir.AluOpType.add,
                )

                # E = exp(-u * softplus(d))
                ed = epi_pool.tile([P, HB], F32, tag="ed")
                nc.scalar.activation(ed, gsb[:, 6 * HB : 7 * HB], AF.Exp)
                spd = epi_pool.tile([P, HB], F32, tag="spd")
                nc.scalar.activation(spd, ed, AF.Ln, bias=1.0)
                E = epi_pool.tile([P, HB], F32, tag="E")
                nc.scalar.activation(E, spd, AF.Exp, scale=negu[:, m : m + 1])

                t1 = epi_pool.tile([P, HB], F32, tag="t1")
                nc.vector.tensor_mul(t1, f1, cx1t)
                t2 = epi_pool.tile([P, HB], F32, tag="t2")
                nc.vector.tensor_mul(t2, i1, z)
                cy1 = out_pool.tile([P, HB], F32, tag="cy1")
                nc.vector.tensor_add(cy1, t1, t2)

                t3 = epi_pool.tile([P, HB], F32, tag="t3")
                nc.vector.tensor_mul(t3, f2, cx2t)
                t4 = epi_pool.tile([P, HB], F32, tag="t4")
                nc.vector.tensor_mul(t4, i2, z)
                cy2 = out_pool.tile([P, HB], F32, tag="cy2")
                nc.vector.tensor_add(cy2, t3, t4)

                dif = epi_pool.tile([P, HB], F32, tag="dif")
                nc.vector.tensor_sub(dif, cy1, cy2)
                t5 = epi_pool.tile([P, HB], F32, tag="t5")
                nc.vector.tensor_mul(t5, dif, E)
                ct = epi_pool.tile([P, HB], F32, tag="ct")
                nc.vector.tensor_add(ct, cy2, t5)

                # tanh(ct) = 2*sigmoid(2*ct) - 1
                ec = epi_pool.tile([P, HB], F32, tag="ec")
                nc.scalar.activation(ec, ct, AF.Exp, scale=-2.0)
                spc = epi_pool.tile([P, HB], F32, tag="spc")
                nc.scalar.activation(spc, ec, AF.Ln, bias=1.0)
                s2c = epi_pool.tile([P, HB], F32, tag="s2c")
                nc.scalar.activation(s2c, spc, AF.Exp, scale=-1.0)
                tct = epi_pool.tile([P, HB], F32, tag="tct")
                nc.vector.tensor_scalar(
                    out=tct, in0=s2c, scalar1=2.0, scalar2=-1.0,
                    op0=mybir.AluOpType.mult, op1=mybir.AluOpType.add,
                )
                ht = out_pool.tile([P, HB], F32, tag="ht")
                nc.vector.tensor_mul(ht, o, tct)

                nc.gpsimd.dma_start(out[0, ms, cs], cy1)
                nc.gpsimd.dma_start(out[1, ms, cs], cy2)
                nc.gpsimd.dma_start(out[2, ms, cs], ht)

    nc.compile()
    return nc


def _get_nc():
    global _cached_nc
    if _cached_nc is None:
        _cached_nc = _build()
    return _cached_nc


def kernel(hx, cx1, cx2, tj, dt, W, b, trace=False):
    nc = _get_nc()
    # W columns permuted [gate, hb, 256] -> [hb, gate, 256], cast bf16
    Wp = np.ascontiguousarray(
        np.asarray(W, dtype=np.float32)
        .reshape(D, NG, N_HB, HB)
        .transpose(0, 2, 1, 3)
        .reshape(D, NG * H)
        .astype(BF16NP)
    )
    bpm = np.ascontiguousarray(
        np.asarray(b, dtype=np.float32)
        .reshape(NG, N_HB, HB)
        .transpose(1, 0, 2)
        .reshape(N_HB, GW)
        .astype(BF16NP)
    )
    hxf = np.asarray(hx, dtype=np.float32)
    in_maps = []
    for c in range(N_CORES):
        rs = slice(c * BL, (c + 1) * BL)
        in_maps.append(
            {
                "hxT": np.ascontiguousarray(hxf[rs].T.astype(BF16NP)),
                "cx1": np.ascontiguousarray(cx1[rs], dtype=np.float32),
                "cx2": np.ascontiguousarray(cx2[rs], dtype=np.float32),
                "dt": np.ascontiguousarray(dt[rs], dtype=np.float32),
                "Wp": Wp,
                "bp": bpm,
            }
        )
    res = bass_utils.run_bass_kernel_spmd(
        nc, in_maps, core_ids=list(range(N_CORES)), trace=trace
    )
    out = np.concatenate([r["out"] for r in res.results], axis=1)
    if trace:
        kernel.last_exec_time_ns = res.exec_time_ns
        kernel.last_results = res
    return out


# revision 4
# speedup vs baseline: 1.2558x; 1.2558x over previous
"""Fused 7-gate continuous-time LSTM cell on 8 Trainium2 NeuronCores.

Data-parallel over the batch dim: each core gets B/8 = 1024 rows, the fused
gate weight W [2048, 7*2048] is replicated (bf16). Per core:
  g = hx @ W + b  with hx/W in bf16 (PE streams 1 col/cycle, FWL hides
  LDWEIGHTS), accumulated fp32 in PSUM.

Layout: W columns are host-permuted to [hb, gate, 256] so each (m-chunk,
hb-block) accumulates all 7 gates' 256 h-columns contiguously (1792 cols =
2 PSUM tiles), letting the epilogue run right after each block.

Epilogue uses only Exp/Ln (single ACT table set, no ~2.7us table switches):
  sigmoid(x) = exp(-softplus(-x)) = Exp(-Ln(1+Exp(-x)))
  tanh(x)    = 2*sigmoid(2x) - 1
  E          = exp(-u*softplus(d)) via Exp(scale=-u[partition]) on Ln(1+Exp(d))
The 5 sigmoid gates are batched as one 1280-wide ACT chain.
"""

import sys

sys.path.insert(0, "/opt/trn_rl_repo")

import numpy as np
import ml_dtypes

import concourse.bass as bass
import concourse.mybir as mybir
import concourse.tile as tile
from concourse import bacc, bass_utils

B, D, H, NG = 8192, 2048, 2048, 7
N_CORES = 8
BL = B // N_CORES  # 1024 rows per core
P = 128
HB = 256  # h-block per epilogue step
N_HB = H // HB  # 8
GW = NG * HB  # 1792 gate-block width per hb
KT = D // P  # 16 contraction subtiles
MT = BL // P  # 8 m-chunks per core
# matmul column slices inside a (m, hb) block: 2 PSUM tiles of [128, 1024]
MM_SLICES = [(0, 0, 512), (0, 512, 512), (1, 0, 512), (1, 512, 256)]

F32 = mybir.dt.float32
BF16 = mybir.dt.bfloat16
AF = mybir.ActivationFunctionType
BF16NP = ml_dtypes.bfloat16

_cached_nc = None

# Make Exp/Ln resolvable only from the one ACT table set that holds both, so
# the table-load inserter hoists a single ACT_TABLE_LOAD instead of thrashing
# ~3 loads (~1.3us each) per block between an exp-set and an ln-set. Set order
# and count are preserved so act_func_set_id indices stay valid.
_ACT_SET = "natural_log_exp_and_others"
_tables_patched = False


def _patch_act_tables():
    global _tables_patched
    if _tables_patched:
        return
    orig = bacc.get_activation_tables

    def patched(arch):
        tabs = {k: set(v) for k, v in orig(arch).items()}
        assert _ACT_SET in tabs and {AF.Exp, AF.Ln} <= tabs[_ACT_SET], tabs.keys()
        for k in tabs:
            if k != _ACT_SET:
                tabs[k] -= {AF.Exp, AF.Ln}
        return tabs

    bacc.get_activation_tables = patched
    _tables_patched = True


def _build():
    _patch_act_tables()
    nc = bacc.Bacc("TRN2", target_bir_lowering=False, debug=False,
                   num_devices=N_CORES)
    hxT = nc.dram_tensor("hxT", [D, BL], BF16, kind="ExternalInput").ap()
    cx1 = nc.dram_tensor("cx1", [BL, H], F32, kind="ExternalInput").ap()
    cx2 = nc.dram_tensor("cx2", [BL, H], F32, kind="ExternalInput").ap()
    dt_in = nc.dram_tensor("dt", [BL, 1], F32, kind="ExternalInput").ap()
    Wp = nc.dram_tensor("Wp", [D, NG * H], BF16, kind="ExternalInput").ap()
    bp = nc.dram_tensor("bp", [N_HB, GW], BF16, kind="ExternalInput").ap()
    out = nc.dram_tensor("out", [3, BL, H], F32, kind="ExternalOutput").ap()

    from contextlib import ExitStack

    with tile.TileContext(nc) as tc, ExitStack() as ctx:
        const_pool = ctx.enter_context(tc.tile_pool(name="const", bufs=1))
        hx_pool = ctx.enter_context(tc.tile_pool(name="hx", bufs=1))
        small_pool = ctx.enter_context(tc.tile_pool(name="small", bufs=4))
        wpool = ctx.enter_context(tc.tile_pool(name="w", bufs=18))
        bias_pool = ctx.enter_context(tc.tile_pool(name="bias", bufs=2))
        psum_pool = ctx.enter_context(tc.tile_pool(name="ps", bufs=4, space="PSUM"))
        gsb_pool = ctx.enter_context(tc.tile_pool(name="gsb", bufs=2))
        sgt_pool = ctx.enter_context(tc.tile_pool(name="sgt", bufs=2))
        sig_pool = ctx.enter_context(tc.tile_pool(name="sig", bufs=2))
        epi_pool = ctx.enter_context(tc.tile_pool(name="epi", bufs=2))
        cx_pool = ctx.enter_context(tc.tile_pool(name="cx", bufs=4))
        out_pool = ctx.enter_context(tc.tile_pool(name="outp", bufs=2))

        # -u per batch row (u = dt), laid out [128, m-chunk]
        negu = const_pool.tile([P, MT], F32)
        for m in range(MT):
            dtt = small_pool.tile([P, 1], F32, tag="dt")
            nc.sync.dma_start(dtt, dt_in[m * P : (m + 1) * P, :])
            nc.vector.tensor_scalar_mul(negu[:, m : m + 1], dtt, -1.0)

        # resident hx^T in bf16: [d-partition, k-subtile, batch-col]
        hxT_sb = hx_pool.tile([P, KT, BL], BF16)
        for k in range(KT):
            nc.gpsimd.dma_start(hxT_sb[:, k, :], hxT[k * P : (k + 1) * P, :])

        for hb in range(N_HB):
            # bias block for this hb, broadcast to 128 partitions
            bsl = bp[hb, :]
            b_bcast = bass.AP(
                tensor=bsl.tensor, offset=bsl.offset, ap=[[0, P], *bsl.ap]
            )
            bt = bias_pool.tile([P, GW], BF16, tag="bt")
            nc.gpsimd.dma_start(bt, b_bcast)

            wts = []
            for k in range(KT):
                wt = wpool.tile([P, GW], BF16, tag="w", name=f"w_{hb}_{k}")
                nc.sync.dma_start(
                    wt, Wp[k * P : (k + 1) * P, hb * GW : (hb + 1) * GW]
                )
                wts.append(wt)

            for m in range(MT):
                ms = slice(m * P, (m + 1) * P)
                cs = slice(hb * HB, (hb + 1) * HB)
                cx1t = cx_pool.tile([P, HB], F32, tag="cx1")
                nc.gpsimd.dma_start(cx1t, cx1[ms, cs])
                cx2t = cx_pool.tile([P, HB], F32, tag="cx2")
                nc.gpsimd.dma_start(cx2t, cx2[ms, cs])

                ps0 = psum_pool.tile([P, 1024], F32, tag="ps", name=f"ps0_{hb}_{m}")
                ps1 = psum_pool.tile([P, 1024], F32, tag="ps", name=f"ps1_{hb}_{m}")
                pst = (ps0, ps1)
                for k in range(KT):
                    lhs = hxT_sb[:, k, ms]
                    for ti, c0, w in MM_SLICES:
                        wc0 = ti * 1024 + c0
                        nc.tensor.matmul(
                            pst[ti][:, c0 : c0 + w],
                            lhs,
                            wts[k][:, wc0 : wc0 + w],
                            start=(k == 0),
                            stop=(k == KT - 1),
                        )

                # ---- epilogue for this [128 rows, 256 h-cols] block ----
                gsb = gsb_pool.tile([P, GW], F32, tag="gsb")
                nc.vector.tensor_add(gsb[:, 0:1024], ps0[:], bt[:, 0:1024])
                nc.vector.tensor_add(gsb[:, 1024:GW], ps1[:, 0:768],
                                     bt[:, 1024:GW])

                # 5 sigmoid gates (i1,i2,f1,f2,o) batched 1280 wide:
                # sig = Exp(-Ln(1 + Exp(-x)))
                e1 = sgt_pool.tile([P, 5 * HB], F32, tag="sgt")
                nc.scalar.activation(e1, gsb[:, 0 : 5 * HB], AF.Exp, scale=-1.0)
                sp = sgt_pool.tile([P, 5 * HB], F32, tag="sgt")
                nc.scalar.activation(sp, e1, AF.Ln, bias=1.0)
                sig = sig_pool.tile([P, 5 * HB], F32, tag="sig")
                nc.scalar.activation(sig, sp, AF.Exp, scale=-1.0)
                i1 = sig[:, 0:HB]
                i2 = sig[:, HB : 2 * HB]
                f1 = sig[:, 2 * HB : 3 * HB]
                f2 = sig[:, 3 * HB : 4 * HB]
                o = sig[:, 4 * HB : 5 * HB]

                # z = tanh(x) = 2*sigmoid(2x) - 1
                ez = epi_pool.tile([P, HB], F32, tag="ez")
                nc.scalar.activation(ez, gsb[:, 5 * HB : 6 * HB], AF.Exp,
                                     scale=-2.0)
                spz = epi_pool.tile([P, HB], F32, tag="spz")
                nc.scalar.activation(spz, ez, AF.Ln, bias=1.0)
                s2z = epi_pool.tile([P, HB], F32, tag="s2z")
                nc.scalar.activation(s2z, spz, AF.Exp, scale=-1.0)
                z = epi_pool.tile([P, HB], F32, tag="z")
                nc.vector.tensor_scalar(
                    out=z, in0=s2z, scalar1=2.0, scalar2=-1.0,
                    op0=mybir.AluOpType.mult, op1=mybir.AluOpType.add,
                )

                # E = exp(-u * softplus(d))
                ed = epi_pool.tile([P, HB], F32, tag="ed")
                nc.scalar.activation(ed, gsb[:, 6 * HB : 7 * HB], AF.Exp)
                spd = epi_pool.tile([P, HB], F32, tag="spd")
                nc.scalar.activation(spd, ed, AF.Ln, bias=1.0)
                E = epi_pool.tile([P, HB], F32, tag="E")
                nc.scalar.activation(E, spd, AF.Exp, scale=negu[:, m : m + 1])

                t1 = epi_pool.tile([P, HB], F32, tag="t1")
                nc.vector.tensor_mul(t1, f1, cx1t)
                t2 = epi_pool.tile([P, HB], F32, tag="t2")
                nc.vector.tensor_mul(t2, i1, z)
                cy1 = out_pool.tile([P, HB], F32, tag="cy1")
                nc.vector.tensor_add(cy1, t1, t2)

                t3 = epi_pool.tile([P, HB], F32, tag="t3")
                nc.vector.tensor_mul(t3, f2, cx2t)
                t4 = epi_pool.tile([P, HB], F32, tag="t4")
                nc.vector.tensor_mul(t4, i2, z)
                cy2 = out_pool.tile([P, HB], F32, tag="cy2")
                nc.vector.tensor_add(cy2, t3, t4)

                dif = epi_pool.tile([P, HB], F32, tag="dif")
                nc.vector.tensor_sub(dif, cy1, cy2)
                t5 = epi_pool.tile([P, HB], F32, tag="t5")
                nc.vector.tensor_mul(t5, dif, E)
                ct = epi_pool.tile([P, HB], F32, tag="ct")
                nc.vector.tensor_add(ct, cy2, t5)

                # tanh(ct) = 2*sigmoid(2*ct) - 1
                ec = epi_pool.tile([P, HB], F32, tag="ec")
                nc.scalar.activation(ec, ct, AF.Exp, scale=-2.0)
                spc = epi_pool.tile([P, HB], F32, tag="spc")
                nc.scalar.activation(spc, ec, AF.Ln, bias=1.0)
                s2c = epi_pool.tile([P, HB], F32, tag="s2c")
                nc.scalar.activation(s2c, spc, AF.Exp, scale=-1.0)
                tct = epi_pool.tile([P, HB], F32, tag="tct")
                nc.vector.tensor_scalar(
                    out=tct, in0=s2c, scalar1=2.0, scalar2=-1.0,
                    op0=mybir.AluOpType.mult, op1=mybir.AluOpType.add,
                )
                ht = out_pool.tile([P, HB], F32, tag="ht")
                nc.vector.tensor_mul(ht, o, tct)

                nc.gpsimd.dma_start(out[0, ms, cs], cy1)
                nc.gpsimd.dma_start(out[1, ms, cs], cy2)
                nc.gpsimd.dma_start(out[2, ms, cs], ht)

    nc.compile()
    return nc


def _get_nc():
    global _cached_nc
    if _cached_nc is None:
        _cached_nc = _build()
    return _cached_nc


def kernel(hx, cx1, cx2, tj, dt, W, b, trace=False):
    nc = _get_nc()
    # W columns permuted [gate, hb, 256] -> [hb, gate, 256], cast bf16
    Wp = np.ascontiguousarray(
        np.asarray(W, dtype=np.float32)
        .reshape(D, NG, N_HB, HB)
        .transpose(0, 2, 1, 3)
        .reshape(D, NG * H)
        .astype(BF16NP)
    )
    bpm = np.ascontiguousarray(
        np.asarray(b, dtype=np.float32)
        .reshape(NG, N_HB, HB)
        .transpose(1, 0, 2)
        .reshape(N_HB, GW)
        .astype(BF16NP)
    )
    hxf = np.asarray(hx, dtype=np.float32)
    in_maps = []
    for c in range(N_CORES):
        rs = slice(c * BL, (c + 1) * BL)
        in_maps.append(
            {
                "hxT": np.ascontiguousarray(hxf[rs].T.astype(BF16NP)),
                "cx1": np.ascontiguousarray(cx1[rs], dtype=np.float32),
                "cx2": np.ascontiguousarray(cx2[rs], dtype=np.float32),
                "dt": np.ascontiguousarray(dt[rs], dtype=np.float32),
                "Wp": Wp,
                "bp": bpm,
            }
        )
    res = bass_utils.run_bass_kernel_spmd(
        nc, in_maps, core_ids=list(range(N_CORES)), trace=trace
    )
    out = np.concatenate([r["out"] for r in res.results], axis=1)
    if trace:
        kernel.last_exec_time_ns = res.exec_time_ns
        kernel.last_results = res
    return out


# revision 7
# speedup vs baseline: 1.4488x; 1.1537x over previous
"""Fused 7-gate continuous-time LSTM cell on 8 Trainium2 NeuronCores.

Data-parallel over the batch dim: each core gets B/8 = 1024 rows, the fused
gate weight W [2048, 7*2048] is replicated (bf16). Per core:
  g = hx @ W + b  with hx/W in bf16 (PE streams 1 col/cycle), fp32 PSUM.

Layout: W columns are host-permuted to [hb, gate, 256] so each (m-chunk,
hb-block) accumulates all 7 gates' 256 h-columns contiguously (1792 cols =
2 PSUM tiles), letting the epilogue run right after each block.

Epilogue uses only Exp/Ln (single ACT table set, no ~2.7us table switches):
  sigmoid(x) = exp(-softplus(-x)) = Exp(-Ln(1+Exp(-x)))
  tanh(x)    = 2*sigmoid(2x) - 1   (z gate W/b pre-scaled x2 on host, so the
                                    z column rides in the batched sigmoid chain)
  E          = exp(-u*softplus(d)) via Exp(scale=-u[partition]) on Ln(1+Exp(d))

The per-block work is software-pipelined in two phases so the in-order DVE
queue never holds block i's gate arithmetic (which waits on ACT) ahead of
block i+1's PSUM-draining bias-add (which ACT i+1 needs):
  phase1(i) = matmuls, bias-add, batched sigmoid chain, E chain
  phase2(i) = gate arithmetic, tanh(c_t) chain, stores   (issued after
              phase1(i+1))
"""

import sys

sys.path.insert(0, "/opt/trn_rl_repo")

import numpy as np
import ml_dtypes

import concourse.bass as bass
import concourse.mybir as mybir
import concourse.tile as tile
from concourse import bacc, bass_utils

B, D, H, NG = 8192, 2048, 2048, 7
N_CORES = 8
BL = B // N_CORES  # 1024 rows per core
P = 128
HB = 256  # h-block per epilogue step
N_HB = H // HB  # 8
GW = NG * HB  # 1792 gate-block width per hb
KT = D // P  # 16 contraction subtiles
MT = BL // P  # 8 m-chunks per core
# matmul column slices inside a (m, hb) block: 2 PSUM tiles of [128, 1024]
MM_SLICES = [(0, 0, 512), (0, 512, 512), (1, 0, 512), (1, 512, 256)]

F32 = mybir.dt.float32
BF16 = mybir.dt.bfloat16
AF = mybir.ActivationFunctionType
BF16NP = ml_dtypes.bfloat16

_cached_nc = None

# Make Exp/Ln resolvable only from the one ACT table set that holds both, so
# the table-load inserter hoists a single ACT_TABLE_LOAD instead of thrashing
# ~3 loads (~1.3us each) per block between an exp-set and an ln-set. Set order
# and count are preserved so act_func_set_id indices stay valid.
_ACT_SET = "natural_log_exp_and_others"
_tables_patched = False


def _patch_act_tables():
    global _tables_patched
    if _tables_patched:
        return
    orig = bacc.get_activation_tables

    def patched(arch):
        tabs = {k: set(v) for k, v in orig(arch).items()}
        assert _ACT_SET in tabs and {AF.Exp, AF.Ln} <= tabs[_ACT_SET], tabs.keys()
        for k in tabs:
            if k != _ACT_SET:
                tabs[k] -= {AF.Exp, AF.Ln}
        return tabs

    bacc.get_activation_tables = patched
    _tables_patched = True


def _build():
    _patch_act_tables()
    nc = bacc.Bacc("TRN2", target_bir_lowering=False, debug=False,
                   num_devices=N_CORES)
    hxT = nc.dram_tensor("hxT", [D, BL], BF16, kind="ExternalInput").ap()
    cx1 = nc.dram_tensor("cx1", [BL, H], F32, kind="ExternalInput").ap()
    cx2 = nc.dram_tensor("cx2", [BL, H], F32, kind="ExternalInput").ap()
    dt_in = nc.dram_tensor("dt", [BL, 1], F32, kind="ExternalInput").ap()
    Wp = nc.dram_tensor("Wp", [D, NG * H], BF16, kind="ExternalInput").ap()
    bp = nc.dram_tensor("bp", [N_HB, GW], BF16, kind="ExternalInput").ap()
    out = nc.dram_tensor("out", [3, BL, H], F32, kind="ExternalOutput").ap()

    from contextlib import ExitStack

    with tile.TileContext(nc) as tc, ExitStack() as ctx:
        const_pool = ctx.enter_context(tc.tile_pool(name="const", bufs=1))
        hx_pool = ctx.enter_context(tc.tile_pool(name="hx", bufs=1))
        small_pool = ctx.enter_context(tc.tile_pool(name="small", bufs=4))
        wpool = ctx.enter_context(tc.tile_pool(name="w", bufs=18))
        bias_pool = ctx.enter_context(tc.tile_pool(name="bias", bufs=2))
        psum_pool = ctx.enter_context(tc.tile_pool(name="ps", bufs=4, space="PSUM"))
        gsb_pool = ctx.enter_context(tc.tile_pool(name="gsb", bufs=2))
        sgt_pool = ctx.enter_context(tc.tile_pool(name="sgt", bufs=2))
        sig_pool = ctx.enter_context(tc.tile_pool(name="sig", bufs=3))
        epi_pool = ctx.enter_context(tc.tile_pool(name="epi", bufs=2))
        cx_pool = ctx.enter_context(tc.tile_pool(name="cx", bufs=4))
        out_pool = ctx.enter_context(tc.tile_pool(name="outp", bufs=2))

        # -u per batch row (u = dt), laid out [128, m-chunk]
        negu = const_pool.tile([P, MT], F32)
        for m in range(MT):
            dtt = small_pool.tile([P, 1], F32, tag="dt")
            nc.sync.dma_start(dtt, dt_in[m * P : (m + 1) * P, :])
            nc.vector.tensor_scalar_mul(negu[:, m : m + 1], dtt, -1.0)

        # resident hx^T in bf16: [d-partition, k-subtile, batch-col]
        hxT_sb = hx_pool.tile([P, KT, BL], BF16)
        for k in range(KT):
            nc.gpsimd.dma_start(hxT_sb[:, k, :], hxT[k * P : (k + 1) * P, :])

        def phase1(hb, m, bt, wts):
            ms = slice(m * P, (m + 1) * P)
            cs = slice(hb * HB, (hb + 1) * HB)
            cx1t = cx_pool.tile([P, HB], F32, tag="cx1")
            nc.gpsimd.dma_start(cx1t, cx1[ms, cs])
            cx2t = cx_pool.tile([P, HB], F32, tag="cx2")
            nc.gpsimd.dma_start(cx2t, cx2[ms, cs])

            ps0 = psum_pool.tile([P, 1024], F32, tag="ps", name=f"ps0_{hb}_{m}")
            ps1 = psum_pool.tile([P, 1024], F32, tag="ps", name=f"ps1_{hb}_{m}")
            pst = (ps0, ps1)
            for k in range(KT):
                lhs = hxT_sb[:, k, ms]
                for ti, c0, w in MM_SLICES:
                    wc0 = ti * 1024 + c0
                    nc.tensor.matmul(
                        pst[ti][:, c0 : c0 + w],
                        lhs,
                        wts[k][:, wc0 : wc0 + w],
                        start=(k == 0),
                        stop=(k == KT - 1),
                    )

            # bias add drains PSUM
            gsb = gsb_pool.tile([P, GW], F32, tag="gsb")
            nc.vector.tensor_add(gsb[:, 0:1024], ps0[:], bt[:, 0:1024])
            nc.vector.tensor_add(gsb[:, 1024:GW], ps1[:, 0:768], bt[:, 1024:GW])

            # 5 sigmoid gates + pre-scaled z gate batched 1536 wide:
            # sig = Exp(-Ln(1 + Exp(-x)))
            e1 = sgt_pool.tile([P, 6 * HB], F32, tag="sgt")
            nc.scalar.activation(e1, gsb[:, 0 : 6 * HB], AF.Exp, scale=-1.0)
            sp = sgt_pool.tile([P, 6 * HB], F32, tag="sgt")
            nc.scalar.activation(sp, e1, AF.Ln, bias=1.0)
            sig = sig_pool.tile([P, 6 * HB], F32, tag="sig")
            nc.scalar.activation(sig, sp, AF.Exp, scale=-1.0)

            # E = exp(-u * softplus(d))
            ed = epi_pool.tile([P, HB], F32, tag="ed")
            nc.scalar.activation(ed, gsb[:, 6 * HB : 7 * HB], AF.Exp)
            spd = epi_pool.tile([P, HB], F32, tag="spd")
            nc.scalar.activation(spd, ed, AF.Ln, bias=1.0)
            E = epi_pool.tile([P, HB], F32, tag="E")
            nc.scalar.activation(E, spd, AF.Exp, scale=negu[:, m : m + 1])

            return dict(ms=ms, cs=cs, cx1t=cx1t, cx2t=cx2t, sig=sig, E=E)

        def phase2(st):
            sig, E = st["sig"], st["E"]
            i1 = sig[:, 0:HB]
            i2 = sig[:, HB : 2 * HB]
            f1 = sig[:, 2 * HB : 3 * HB]
            f2 = sig[:, 3 * HB : 4 * HB]
            o = sig[:, 4 * HB : 5 * HB]
            # z = tanh = 2*sigmoid(2x)-1 (the x2 was folded into W/b on host)
            z = epi_pool.tile([P, HB], F32, tag="z")
            nc.vector.tensor_scalar(
                out=z, in0=sig[:, 5 * HB : 6 * HB], scalar1=2.0, scalar2=-1.0,
                op0=mybir.AluOpType.mult, op1=mybir.AluOpType.add,
            )

            t1 = epi_pool.tile([P, HB], F32, tag="t1")
            nc.vector.tensor_mul(t1, f1, st["cx1t"])
            t2 = epi_pool.tile([P, HB], F32, tag="t2")
            nc.vector.tensor_mul(t2, i1, z)
            cy1 = out_pool.tile([P, HB], F32, tag="cy1")
            nc.vector.tensor_add(cy1, t1, t2)

            t3 = epi_pool.tile([P, HB], F32, tag="t3")
            nc.vector.tensor_mul(t3, f2, st["cx2t"])
            t4 = epi_pool.tile([P, HB], F32, tag="t4")
            nc.vector.tensor_mul(t4, i2, z)
            cy2 = out_pool.tile([P, HB], F32, tag="cy2")
            nc.vector.tensor_add(cy2, t3, t4)

            dif = epi_pool.tile([P, HB], F32, tag="dif")
            nc.vector.tensor_sub(dif, cy1, cy2)
            t5 = epi_pool.tile([P, HB], F32, tag="t5")
            nc.vector.tensor_mul(t5, dif, E)
            ct = epi_pool.tile([P, HB], F32, tag="ct")
            nc.vector.tensor_add(ct, cy2, t5)

            # tanh(ct) = 2*sigmoid(2*ct) - 1
            ec = epi_pool.tile([P, HB], F32, tag="ec")
            nc.scalar.activation(ec, ct, AF.Exp, scale=-2.0)
            spc = epi_pool.tile([P, HB], F32, tag="spc")
            nc.scalar.activation(spc, ec, AF.Ln, bias=1.0)
            s2c = epi_pool.tile([P, HB], F32, tag="s2c")
            nc.scalar.activation(s2c, spc, AF.Exp, scale=-1.0)
            tct = epi_pool.tile([P, HB], F32, tag="tct")
            nc.vector.tensor_scalar(
                out=tct, in0=s2c, scalar1=2.0, scalar2=-1.0,
                op0=mybir.AluOpType.mult, op1=mybir.AluOpType.add,
            )
            ht = out_pool.tile([P, HB], F32, tag="ht")
            nc.vector.tensor_mul(ht, o, tct)

            nc.gpsimd.dma_start(out[0, st["ms"], st["cs"]], cy1)
            nc.gpsimd.dma_start(out[1, st["ms"], st["cs"]], cy2)
            nc.gpsimd.dma_start(out[2, st["ms"], st["cs"]], ht)

        pending = None
        for hb in range(N_HB):
            # bias block for this hb, broadcast to 128 partitions
            bsl = bp[hb, :]
            b_bcast = bass.AP(
                tensor=bsl.tensor, offset=bsl.offset, ap=[[0, P], *bsl.ap]
            )
            bt = bias_pool.tile([P, GW], BF16, tag="bt")
            nc.gpsimd.dma_start(bt, b_bcast)

            wts = []
            for k in range(KT):
                wt = wpool.tile([P, GW], BF16, tag="w", name=f"w_{hb}_{k}")
                nc.sync.dma_start(
                    wt, Wp[k * P : (k + 1) * P, hb * GW : (hb + 1) * GW]
                )
                wts.append(wt)

            for m in range(MT):
                st = phase1(hb, m, bt, wts)
                if pending is not None:
                    phase2(pending)
                pending = st
        phase2(pending)

    nc.compile()
    return nc


def _get_nc():
    global _cached_nc
    if _cached_nc is None:
        _cached_nc = _build()
    return _cached_nc


def kernel(hx, cx1, cx2, tj, dt, W, b, trace=False):
    nc = _get_nc()
    Wm = np.asarray(W, dtype=np.float32).copy()
    bm = np.asarray(b, dtype=np.float32).reshape(NG * H).copy()
    # fold tanh(x) = 2*sigmoid(2x)-1: pre-scale z-gate columns by 2
    Wm[:, 5 * H : 6 * H] *= 2.0
    bm[5 * H : 6 * H] *= 2.0
    # W columns permuted [gate, hb, 256] -> [hb, gate, 256], cast bf16
    Wp = np.ascontiguousarray(
        Wm.reshape(D, NG, N_HB, HB)
        .transpose(0, 2, 1, 3)
        .reshape(D, NG * H)
        .astype(BF16NP)
    )
    bpm = np.ascontiguousarray(
        bm.reshape(NG, N_HB, HB).transpose(1, 0, 2).reshape(N_HB, GW).astype(BF16NP)
    )
    hxf = np.asarray(hx, dtype=np.float32)
    in_maps = []
    for c in range(N_CORES):
        rs = slice(c * BL, (c + 1) * BL)
        in_maps.append(
            {
                "hxT": np.ascontiguousarray(hxf[rs].T.astype(BF16NP)),
                "cx1": np.ascontiguousarray(cx1[rs], dtype=np.float32),
                "cx2": np.ascontiguousarray(cx2[rs], dtype=np.float32),
                "dt": np.ascontiguousarray(dt[rs], dtype=np.float32),
                "Wp": Wp,
                "bp": bpm,
            }
        )
    res = bass_utils.run_bass_kernel_spmd(
        nc, in_maps, core_ids=list(range(N_CORES)), trace=trace
    )
    out = np.concatenate([r["out"] for r in res.results], axis=1)
    if trace:
        kernel.last_exec_time_ns = res.exec_time_ns
        kernel.last_results = res
    return out
